# revision 1
# baseline (speedup 1.0000x reference)
"""DMPNN last layer on 8 Trainium2 NeuronCores.

out = relu(concat([x, segment_sum(h, edge_dst, N)], 1) @ W.T + b)

Strategy (graph-parallel, no collectives):
  - Host partitions the 40000 nodes into 8 contiguous ranges of 5000 and
    routes every edge (its h row + dst index) to the core owning dst.
  - Per core, edges are grouped by 128-node tile of their dst and each
    group is padded to a multiple of 128 edges (pad edges carry dst=-10000
    so they never match). Group chunk counts are equalized across cores so
    all 8 cores run the same program.
  - Device: for each 128-node tile, chunks of 128 edges are loaded as
    [128 edge-partitions x 128 feat] tiles; a one-hot matrix
    onehot[e, n] = (dst_local[e] == n) is built with iota + is_equal and
    used as the moving operand of a matmul with the h chunk as stationary:
    PSUM[hid, n] += h_chunk.T @ onehot  ==  segment-sum, transposed.
    Two more matmuls apply the linear layer (weights pre-transposed on
    host), and ScalarE does relu + bias. The output is produced
    transposed ([128 out-feat x nodes]); the host transposes it back.
"""

import os
from contextlib import ExitStack

import numpy as np
import ml_dtypes

import bass_rust
import concourse.bass as bass
import concourse.mybir as mybir
import concourse.tile as tile
from concourse.vector_clock import ScopedClock
from concourse.bass_utils import run_bass_kernel_spmd

N_NODES = 40000
N_EDGES = 640000
F = 128
HID = 128
N_CORES = 8
NPC = N_NODES // N_CORES  # nodes per core
P = 128
NT = (NPC + P - 1) // P  # node tiles per core
NPAD = NT * P  # padded nodes per core

# h / one-hot compute dtype. bf16 halves the dominant DMA stream and is
# exact for the one-hot compare (integers 0..127 are exact in bf16).
H_BF16 = os.environ.get("KERNEL_H_DTYPE", "bf16") == "bf16"
H_DT = mybir.dt.bfloat16 if H_BF16 else mybir.dt.float32
H_NP = ml_dtypes.bfloat16 if H_BF16 else np.float32

PAD_DST = -10000.0  # never equals an iota value in [0, 128)


# This walrus build rejects more than MAX_WAITS sem waits on a single
# instruction. Post-pass: hoist excess waits onto same-engine nops inserted
# just before the offending instruction (same-engine program order keeps the
# semantics: wait-all split across sequential instructions).
MAX_WAITS = 1
_split_cnt = [0]


def _split_excess_waits(nc, max_waits=MAX_WAITS):
    for fn in nc.m.functions:
        for bb in fn.blocks:
            out = []
            changed = False
            for inst in bb.instructions:
                si = inst.sync_info
                waits = list(si.on_wait) if si is not None and si.on_wait else []
                if len(waits) > max_waits:
                    changed = True
                    head, keep = waits[:-max_waits], waits[-max_waits:]
                    for j in range(0, len(head), max_waits):
                        _split_cnt[0] += 1
                        nop = mybir.InstNoOp(
                            name=f"SWSPLIT-{_split_cnt[0]}", ins=[], outs=[]
                        )
                        nop.engine = inst.engine
                        nop.sync_info = bass_rust.SyncInfo(
                            on_wait=head[j : j + max_waits], on_update=[]
                        )
                        out.append(nop)
                    inst.sync_info = bass_rust.SyncInfo(
                        on_wait=keep, on_update=si.on_update
                    )
                out.append(inst)
            if changed:
                bb.instructions = out


def preprocess(x, h, edge_dst, W, b):
    """Route/sort/pad inputs into one in_map per core. Returns
    (in_maps, C, node_map): C[t] is the (core-uniform) chunk count of node
    tile t; node_map[c][slot] is the global node id at that output slot
    (-1 for unused padding slots). Contiguous node ranges per core: a
    degree-balanced snake permutation was tried and measured ~7us/body
    SLOWER on HW despite 3.6% fewer bytes (paired A/B at 65 reps)."""
    x = np.asarray(x, dtype=np.float32)
    h = np.asarray(h, dtype=np.float32)
    W = np.asarray(W, dtype=np.float32)
    b = np.asarray(b, dtype=np.float32)
    dst = np.asarray(edge_dst).astype(np.int64)

    idx = np.arange(N_NODES, dtype=np.int64)
    node_core = idx // NPC
    node_slot = idx - node_core * NPC  # local slot within the core

    node_map = np.full((N_CORES, NPAD), -1, dtype=np.int64)
    node_map[node_core, node_slot] = idx

    core = node_core[dst]
    dstl = node_slot[dst]  # local node slot in [0, NPAD)
    t_id = dstl >> 7  # node-tile id in [0, NT)
    g = core * NT + t_id  # global group id
    order = np.argsort(g, kind="stable")
    gs = g[order]
    counts = np.bincount(g, minlength=N_CORES * NT).reshape(N_CORES, NT)
    C = np.maximum(1, -(-counts // P)).max(axis=0)  # [NT] chunks per tile
    Ctot = int(C.sum())
    EPAD = Ctot * P

    tile_base = np.zeros(NT, dtype=np.int64)
    tile_base[1:] = np.cumsum(C[:-1]) * P
    grp_start = np.zeros(N_CORES * NT + 1, dtype=np.int64)
    grp_start[1:] = np.cumsum(counts.reshape(-1))
    rank = np.arange(N_EDGES, dtype=np.int64) - grp_start[gs]
    slot = tile_base[gs % NT] + rank  # position in the core's padded arrays
    core_s = gs // NT

    # h laid out exactly as the SBUF tiles consume it: [partition(=edge%128),
    # global chunk, feat] so every DMA descriptor is a contiguous 128*HID run.
    h_s = np.zeros((N_CORES, P, Ctot, HID), dtype=H_NP)
    h_s[core_s, slot % P, slot // P] = h[order].astype(H_NP)
    h_s = h_s.reshape(N_CORES, P, Ctot * HID)

    dst_pre = np.full((N_CORES, EPAD), PAD_DST, dtype=np.float32)
    dst_pre[core_s, slot] = (dstl[order] - (gs % NT) * P).astype(np.float32)
    # dstm[c][p][j] = dst_pre[c][j*128 + p]
    dstm = np.ascontiguousarray(dst_pre.reshape(N_CORES, Ctot, P).transpose(0, 2, 1))

    xT = np.zeros((N_CORES, F, NPAD), dtype=np.float32)
    xT[:, :, :NPC] = x.reshape(N_CORES, NPC, F).transpose(0, 2, 1)

    wT = np.ascontiguousarray(W.T)  # [F+HID, HID]
    bias = np.ascontiguousarray(b.reshape(HID, 1))

    in_maps = [
        {
            "hs": np.ascontiguousarray(h_s[c]),
            "dstm": dstm[c],
            "xT": np.ascontiguousarray(xT[c]),
            "wT": wT,
            "bias": bias,
        }
        for c in range(N_CORES)
    ]
    return in_maps, [int(v) for v in C], node_map


def build(C, reps=1):
    Ctot = sum(C)
    EPAD = Ctot * P
    CMAX = max(C)
    f32 = mybir.dt.float32

    nc = bass.Bass()
    hs = nc.dram_tensor("hs", [P, Ctot * HID], H_DT, kind="ExternalInput")
    dstm = nc.dram_tensor("dstm", [P, Ctot], f32, kind="ExternalInput")
    xT = nc.dram_tensor("xT", [F, NPAD], f32, kind="ExternalInput")
    wT = nc.dram_tensor("wT", [F + HID, HID], f32, kind="ExternalInput")
    bias = nc.dram_tensor("bias", [HID, 1], f32, kind="ExternalInput")
    outT = nc.dram_tensor("outT", [HID, NPAD], f32, kind="ExternalOutput")

    with tile.TileContext(nc) as tc, ExitStack() as ctx:
        const = ctx.enter_context(tc.tile_pool(name="const", bufs=1))
        hpool = ctx.enter_context(tc.tile_pool(name="hpool", bufs=4))
        ohpool = ctx.enter_context(tc.tile_pool(name="ohpool", bufs=3))
        xpool = ctx.enter_context(tc.tile_pool(name="xpool", bufs=3))
        hapool = ctx.enter_context(tc.tile_pool(name="hapool", bufs=2))
        opool = ctx.enter_context(tc.tile_pool(name="opool", bufs=3))
        psh = ctx.enter_context(tc.tile_pool(name="psh", bufs=3, space="PSUM"))
        pso = ctx.enter_context(tc.tile_pool(name="pso", bufs=3, space="PSUM"))

        iota_i = const.tile([P, P], mybir.dt.int32)
        nc.gpsimd.iota(iota_i[:], pattern=[[1, P]], base=0, channel_multiplier=0)
        iota_c = const.tile([P, P], H_DT)
        nc.vector.tensor_copy(iota_c[:], iota_i[:])

        dstm_sb = const.tile([P, Ctot], f32)
        nc.sync.dma_start(dstm_sb[:], dstm[:])
        wx = const.tile([P, HID], f32)
        nc.sync.dma_start(wx[:], wT[0:F, :])
        wh = const.tile([P, HID], f32)
        nc.sync.dma_start(wh[:], wT[F : F + HID, :])
        bt = const.tile([P, 1], f32)
        nc.sync.dma_start(bt[:], bias[:])

        for _rep in range(reps):
            j0 = 0
            for t in range(NT):
                Ct = C[t]
                h_t = hpool.tile([P, Ct * P], H_DT, tag="h_t")
                # Alternate the big h stream between the two HWDGE rings (SP
                # and ACT sequencers); x/out ride whichever ring h is not on.
                heng = nc.sync
                oeng = nc.scalar
                heng.dma_start(h_t[:], hs[:, j0 * P : (j0 + Ct) * P])
                oh = ohpool.tile([P, Ct * P], H_DT, tag="oh")
                for k in range(Ct):
                    # onehot row e = (iota == dst[e]): per-partition scalar
                    # compare; unit-stride bf16 operands run DVE at 4x.
                    nc.vector.tensor_scalar(
                        out=oh[:, k * P : (k + 1) * P],
                        in0=iota_c[:],
                        scalar1=dstm_sb[:, j0 + k : j0 + k + 1],
                        scalar2=None,
                        op0=mybir.AluOpType.is_equal,
                    )
                ph = psh.tile([P, P], mybir.dt.float32, tag="ph")
                for k in range(Ct):
                    nc.tensor.matmul(
                        out=ph[:],
                        lhsT=h_t[:, k * P : (k + 1) * P],
                        rhs=oh[:, k * P : (k + 1) * P],
                        start=(k == 0),
                        stop=(k == Ct - 1),
                    )
                hA = hapool.tile([P, P], f32, tag="hA")
                nc.scalar.copy(hA[:], ph[:])

                xt = xpool.tile([P, P], f32, tag="xt")
                oeng.dma_start(xt[:], xT[:, t * P : (t + 1) * P])
                po = pso.tile([P, P], mybir.dt.float32, tag="po")
                nc.tensor.matmul(out=po[:], lhsT=wx[:], rhs=xt[:], start=True, stop=False)
                nc.tensor.matmul(out=po[:], lhsT=wh[:], rhs=hA[:], start=False, stop=True)
                ob = opool.tile([P, P], f32, tag="ob")
                nc.scalar.activation(
                    ob[:], po[:], mybir.ActivationFunctionType.Relu, bias=bt[:, :1]
                )
                oeng.dma_start(outT[:, t * P : (t + 1) * P], ob[:])
                j0 += Ct
    return nc


def postprocess(results, node_map):
    out = np.empty((N_NODES, HID), dtype=np.float32)
    for c in range(N_CORES):
        ids = node_map[c]
        mask = ids >= 0
        out[ids[mask]] = results[c]["outT"].T[mask]
    return out


def kernel(x, h, edge_dst, W, b, **_kw):
    in_maps, C, node_map = preprocess(x, h, edge_dst, W, b)
    nc = build(C)
    _split_excess_waits(nc)  # HW-only pass (the sim race detector rejects it)
    results = None
    last_err = None
    for _attempt in range(3):  # device occasionally reports a transient
        try:  # NRT_EXEC_UNIT_UNRECOVERABLE right after a heavy prior session
            res = run_bass_kernel_spmd(nc, in_maps, list(range(N_CORES)))
            results = res.results
            break
        except ModuleNotFoundError:
            # trace path needs antenv.axon_hooks, absent in trimmed clients
            from concourse import bass2jax

            results = bass2jax.run_bass_via_pjrt(nc, in_maps, n_cores=N_CORES)
            break
        except Exception as e:  # noqa: BLE001
            last_err = e
            if "UNRECOVERABLE" not in str(e) and "UNAVAILABLE" not in str(e):
                raise
            import time as _time

            _time.sleep(10)
    if results is None:
        raise last_err
    return postprocess(results, node_map)



# revision 4
# speedup vs baseline: 1.1232x; 1.1232x over previous
"""DMPNN last layer on 8 Trainium2 NeuronCores.

out = relu(concat([x, segment_sum(h, edge_dst, N)], 1) @ W.T + b)

Strategy (graph-parallel, no collectives):
  - Host partitions the 40000 nodes across 8 cores (5000 each) and routes
    every edge (its h row + dst index) to the core owning dst.
  - Nodes are degree-balanced: a snake deal over degree-sorted nodes
    equalizes per-core edge counts, then a cap-aware deal packs each
    core's nodes into 40 tiles of 128 nodes whose edge counts fit a
    global per-tile chunk budget C[t] (shared by all cores, so one
    program serves all 8). This cuts one-hot chunk padding to <1%.
  - Device: for each 128-node tile, chunks of 128 edges are loaded as
    [128 edge-partitions x 128 feat] bf16 tiles; a one-hot matrix
    onehot[e, n] = (dst_local[e] == n) is built with iota + is_equal and
    used as the moving operand of a matmul with the h chunk stationary:
    PSUM[hid, n] += h_chunk.T @ onehot  ==  segment-sum, transposed.
    Two more matmuls apply the linear layer, ScalarE does relu + bias.
  - All tensors ride in bf16 (h, x, dst indices, W, output) to halve
    HBM traffic; PSUM accumulation stays fp32. The output is produced
    transposed ([128 out-feat x nodes]); the host transposes it back.
"""

import os
from contextlib import ExitStack

import numpy as np
import ml_dtypes

import bass_rust
import concourse.bass as bass
import concourse.mybir as mybir
import concourse.tile as tile
from concourse.bass_utils import run_bass_kernel_spmd

N_NODES = 40000
N_EDGES = 640000
F = 128
HID = 128
N_CORES = 8
NPC = N_NODES // N_CORES  # nodes per core
P = 128
NT = (NPC + P - 1) // P  # node tiles per core
NPAD = NT * P  # padded nodes per core

BF16 = mybir.dt.bfloat16
NP_BF16 = ml_dtypes.bfloat16

# Host-side routing mode: "binpack" (degree-balanced tiles, minimal chunk
# padding) or "contig" (contiguous node ranges, baseline-style).
ROUTE_MODE = os.environ.get("KERNEL_ROUTE", "binpack")
# Output-store granularity (tiles per output DMA).
OCHUNK = int(os.environ.get("KERNEL_OCHUNK", "10"))


# This walrus build rejects more than MAX_WAITS sem waits on a single
# instruction. Post-pass: hoist excess waits onto same-engine nops inserted
# just before the offending instruction (same-engine program order keeps the
# semantics: wait-all split across sequential instructions).
MAX_WAITS = 1
_split_cnt = [0]


def _split_excess_waits(nc, max_waits=MAX_WAITS):
    for fn in nc.m.functions:
        for bb in fn.blocks:
            out = []
            changed = False
            for inst in bb.instructions:
                si = inst.sync_info
                waits = list(si.on_wait) if si is not None and si.on_wait else []
                if len(waits) > max_waits:
                    changed = True
                    head, keep = waits[:-max_waits], waits[-max_waits:]
                    for j in range(0, len(head), max_waits):
                        _split_cnt[0] += 1
                        nop = mybir.InstNoOp(
                            name=f"SWSPLIT-{_split_cnt[0]}", ins=[], outs=[]
                        )
                        nop.engine = inst.engine
                        nop.sync_info = bass_rust.SyncInfo(
                            on_wait=head[j : j + max_waits], on_update=[]
                        )
                        out.append(nop)
                    inst.sync_info = bass_rust.SyncInfo(
                        on_wait=keep, on_update=si.on_update
                    )
                out.append(inst)
            if changed:
                bb.instructions = out


def _route_nodes(deg):
    """Assign nodes to (core, tile) and pick the global chunk budget C.

    Returns (node_core, node_slot, C) with node_slot in [0, NPAD); C[t] is
    the chunk count of tile t, identical for every core."""
    if ROUTE_MODE == "contig":
        idx = np.arange(N_NODES, dtype=np.int64)
        node_core = idx // NPC
        node_slot = idx - node_core * NPC
        counts = np.zeros((N_CORES, NT), dtype=np.int64)
        np.add.at(counts, (node_core, node_slot >> 7), deg)
        C = np.maximum(1, -(-counts // P)).max(axis=0)
        return node_core, node_slot, [int(v) for v in C]

    # Snake deal over degree-sorted nodes -> equal per-core edge totals.
    order = np.argsort(-deg, kind="stable")
    snake = np.empty((NPC, N_CORES), dtype=np.int64)
    fwd = np.arange(N_CORES)
    for r in range(NPC):
        snake[r] = fwd if r % 2 == 0 else fwd[::-1]
    core_of = np.empty(N_NODES, dtype=np.int64)
    core_of[order] = snake.reshape(-1)

    ecore = np.zeros(N_CORES, dtype=np.int64)
    np.add.at(ecore, core_of, deg)

    node_core = core_of
    node_slot = np.empty(N_NODES, dtype=np.int64)

    csum0 = int(-(-ecore.max() // P))
    for csum in range(csum0, csum0 + 16):
        base, rem = divmod(csum, NT)
        caps = np.full(NT, base, dtype=np.int64) * P
        caps[:rem] += P  # tiles [0, rem) get one extra chunk
        ok = True
        slots_all = []
        for c in range(N_CORES):
            nodes = np.where(core_of == c)[0]
            nd = deg[nodes]
            o = np.argsort(-nd, kind="stable")
            nodes, nd = nodes[o], nd[o]
            # proportional fill: each node (desc by degree) goes to the tile
            # with the largest remaining per-slot target deficit.
            tau = caps * (nd.sum() / caps.sum())
            load = np.zeros(NT, dtype=np.float64)
            nslots = np.full(NT, P, dtype=np.int64)
            tile_of = np.empty(NPC, dtype=np.int64)
            rank_of = np.empty(NPC, dtype=np.int64)
            for i in range(NPC):
                score = np.where(
                    nslots > 0, (tau - load) / np.maximum(nslots, 1), -1e18
                )
                t = int(np.argmax(score))
                tile_of[i] = t
                rank_of[i] = P - nslots[t]
                load[t] += nd[i]
                nslots[t] -= 1
            if (load.astype(np.int64) > caps).any():
                ok = False
                break
            slots_all.append((nodes, tile_of * P + rank_of))
        if ok:
            for c in range(N_CORES):
                nodes, slots = slots_all[c]
                node_slot[nodes] = slots
            C = [int(caps[t]) // P for t in range(NT)]
            return node_core, node_slot, C
    raise RuntimeError("binpack failed")


def preprocess(x, h, edge_dst, W, b):
    """Route/sort/pad inputs into one in_map per core. Returns
    (in_maps, C, node_map): C[t] is the (core-uniform) chunk count of node
    tile t; node_map[c][slot] is the global node id at that output slot
    (-1 for unused padding slots)."""
    x = np.asarray(x, dtype=np.float32)
    h = np.asarray(h, dtype=np.float32)
    W = np.asarray(W, dtype=np.float32)
    b = np.asarray(b, dtype=np.float32)
    dst = np.asarray(edge_dst).astype(np.int64)

    deg = np.bincount(dst, minlength=N_NODES)
    node_core, node_slot, C = _route_nodes(deg)

    node_map = np.full((N_CORES, NPAD), -1, dtype=np.int64)
    node_map[node_core, node_slot] = np.arange(N_NODES, dtype=np.int64)

    core = node_core[dst]
    dstl = node_slot[dst]  # local node slot in [0, NPAD)
    t_id = dstl >> 7  # node-tile id in [0, NT)
    g = core * NT + t_id  # global group id
    order = np.argsort(g, kind="stable")
    gs = g[order]
    counts = np.bincount(g, minlength=N_CORES * NT).reshape(N_CORES, NT)
    Ca = np.asarray(C, dtype=np.int64)
    assert (counts <= Ca[None, :] * P).all()
    Ctot = int(Ca.sum())
    EPAD = Ctot * P

    tile_base = np.zeros(NT, dtype=np.int64)
    tile_base[1:] = np.cumsum(Ca[:-1]) * P
    grp_start = np.zeros(N_CORES * NT + 1, dtype=np.int64)
    grp_start[1:] = np.cumsum(counts.reshape(-1))
    rank = np.arange(N_EDGES, dtype=np.int64) - grp_start[gs]
    slot = tile_base[gs % NT] + rank  # position in the core's padded arrays
    core_s = gs // NT

    # h laid out exactly as the SBUF tiles consume it: [partition(=edge%128),
    # global chunk, feat] so every DMA descriptor is a contiguous 128*HID run.
    h_s = np.zeros((N_CORES, P, Ctot, HID), dtype=NP_BF16)
    h_s[core_s, slot % P, slot // P] = h[order].astype(NP_BF16)
    h_s = h_s.reshape(N_CORES, P, Ctot * HID)

    # pad edges carry dst_rel = 0; their h rows are zero so they contribute
    # nothing to node 0 of their tile.
    dst_pre = np.zeros((N_CORES, EPAD), dtype=np.float32)
    dst_pre[core_s, slot] = (dstl[order] - (gs % NT) * P).astype(np.float32)
    # dstm[c][p][j] = dst_pre[c][j*128 + p]
    dstm = np.ascontiguousarray(dst_pre.reshape(N_CORES, Ctot, P).transpose(0, 2, 1))

    xT = np.zeros((N_CORES, F, NPAD), dtype=NP_BF16)
    for c in range(N_CORES):
        ids = node_map[c]
        m = ids >= 0
        xT[c][:, m] = x[ids[m]].astype(NP_BF16).T

    wT = np.ascontiguousarray(W.T.astype(NP_BF16))  # [F+HID, HID]
    bias = np.ascontiguousarray(b.reshape(HID, 1))

    in_maps = [
        {
            "hs": np.ascontiguousarray(h_s[c]),
            "dstm": dstm[c],
            "xT": np.ascontiguousarray(xT[c]),
            "wT": wT,
            "bias": bias,
        }
        for c in range(N_CORES)
    ]
    return in_maps, C, node_map


def build(C, reps=1, loop_reps=1):
    Ctot = sum(C)
    f32 = mybir.dt.float32

    nc = bass.Bass()
    hs = nc.dram_tensor("hs", [P, Ctot * HID], BF16, kind="ExternalInput")
    dstm = nc.dram_tensor("dstm", [P, Ctot], f32, kind="ExternalInput")
    xT = nc.dram_tensor("xT", [F, NPAD], BF16, kind="ExternalInput")
    wT = nc.dram_tensor("wT", [F + HID, HID], BF16, kind="ExternalInput")
    bias = nc.dram_tensor("bias", [HID, 1], f32, kind="ExternalInput")
    outT = nc.dram_tensor("outT", [HID, NPAD], BF16, kind="ExternalOutput")

    with tile.TileContext(nc) as tc, ExitStack() as ctx:
        const = ctx.enter_context(tc.tile_pool(name="const", bufs=1))
        inpool = ctx.enter_context(tc.tile_pool(name="inpool", bufs=1))
        hpool = ctx.enter_context(tc.tile_pool(name="hpool", bufs=4))
        ohpool = ctx.enter_context(tc.tile_pool(name="ohpool", bufs=4))
        hapool = ctx.enter_context(tc.tile_pool(name="hapool", bufs=2))
        opool = ctx.enter_context(tc.tile_pool(name="opool", bufs=1))
        psh = ctx.enter_context(tc.tile_pool(name="psh", bufs=4, space="PSUM"))
        pso = ctx.enter_context(tc.tile_pool(name="pso", bufs=4, space="PSUM"))

        iota_i = const.tile([P, P], mybir.dt.int32)
        nc.gpsimd.iota(iota_i[:], pattern=[[1, P]], base=0, channel_multiplier=0)
        iota_c = const.tile([P, P], BF16)
        nc.vector.tensor_copy(iota_c[:], iota_i[:])

        def body():
            # Per-execution input loads: small tensors ride the ACT HWDGE
            # ring; the big h stream has the SP ring to itself.
            dstm_sb = inpool.tile([P, Ctot], f32, tag="dstm")
            nc.scalar.dma_start(dstm_sb[:], dstm[:])
            wx = inpool.tile([P, HID], BF16, tag="wx")
            nc.scalar.dma_start(wx[:], wT[0:F, :])
            wh = inpool.tile([P, HID], BF16, tag="wh")
            nc.scalar.dma_start(wh[:], wT[F : F + HID, :])
            bt = inpool.tile([P, 1], f32, tag="bt")
            nc.scalar.dma_start(bt[:], bias[:])
            xall = inpool.tile([P, NPAD], BF16, tag="xall")
            nc.scalar.dma_start(xall[:], xT[:])
            obuf = opool.tile([P, NPAD], BF16, tag="obuf")

            j0 = 0
            for t in range(NT):
                Ct = C[t]
                h_t = hpool.tile([P, Ct * HID], BF16, tag="h_t")
                nc.sync.dma_start(h_t[:], hs[:, j0 * HID : (j0 + Ct) * HID])
                oh = ohpool.tile([P, Ct * P], BF16, tag="oh")
                for k in range(Ct):
                    # onehot row e = (iota == dst[e]): per-partition scalar
                    # compare; unit-stride bf16 operands run DVE at 4x.
                    nc.vector.tensor_scalar(
                        out=oh[:, k * P : (k + 1) * P],
                        in0=iota_c[:],
                        scalar1=dstm_sb[:, j0 + k : j0 + k + 1],
                        scalar2=None,
                        op0=mybir.AluOpType.is_equal,
                    )
                ph = psh.tile([P, P], f32, tag="ph")
                for k in range(Ct):
                    nc.tensor.matmul(
                        out=ph[:],
                        lhsT=h_t[:, k * HID : (k + 1) * HID],
                        rhs=oh[:, k * P : (k + 1) * P],
                        start=(k == 0),
                        stop=(k == Ct - 1),
                    )
                hA = hapool.tile([P, P], BF16, tag="hA")
                nc.scalar.copy(hA[:], ph[:])

                po = pso.tile([P, P], f32, tag="po")
                nc.tensor.matmul(
                    out=po[:],
                    lhsT=wx[:],
                    rhs=xall[:, t * P : (t + 1) * P],
                    start=True,
                    stop=False,
                )
                nc.tensor.matmul(out=po[:], lhsT=wh[:], rhs=hA[:], start=False, stop=True)
                nc.scalar.activation(
                    obuf[:, t * P : (t + 1) * P],
                    po[:],
                    mybir.ActivationFunctionType.Relu,
                    bias=bt[:, :1],
                )
                if (t + 1) % OCHUNK == 0 or t == NT - 1:
                    lo = (t // OCHUNK) * OCHUNK
                    nc.scalar.dma_start(
                        outT[:, lo * P : (t + 1) * P], obuf[:, lo * P : (t + 1) * P]
                    )
                j0 += Ct

        for _rep in range(reps):
            if loop_reps > 1:
                with tc.For_i(0, loop_reps):
                    body()
            else:
                body()
    return nc


def postprocess(results, node_map):
    out = np.empty((N_NODES, HID), dtype=np.float32)
    for c in range(N_CORES):
        ids = node_map[c]
        mask = ids >= 0
        out[ids[mask]] = results[c]["outT"].astype(np.float32).T[mask]
    return out


def kernel(x, h, edge_dst, W, b, **_kw):
    in_maps, C, node_map = preprocess(x, h, edge_dst, W, b)
    nc = build(C)
    _split_excess_waits(nc)  # HW-only pass (the sim race detector rejects it)
    results = None
    last_err = None
    for _attempt in range(3):  # device occasionally reports a transient
        try:  # NRT_EXEC_UNIT_UNRECOVERABLE right after a heavy prior session
            res = run_bass_kernel_spmd(nc, in_maps, list(range(N_CORES)))
            results = res.results
            break
        except ModuleNotFoundError:
            # trace path needs antenv.axon_hooks, absent in trimmed clients
            from concourse import bass2jax

            results = bass2jax.run_bass_via_pjrt(nc, in_maps, n_cores=N_CORES)
            break
        except Exception as e:  # noqa: BLE001
            last_err = e
            if "UNRECOVERABLE" not in str(e) and "UNAVAILABLE" not in str(e):
                raise
            import time as _time

            _time.sleep(10)
    if results is None:
        raise last_err
    return postprocess(results, node_map)


# revision 9
# speedup vs baseline: 1.7464x; 1.5548x over previous
"""DMPNN last layer on 8 Trainium2 NeuronCores.

out = relu(concat([x, segment_sum(h, edge_dst, N)], 1) @ W.T + b)

Strategy (graph-parallel, no collectives):
  - Host partitions the 40000 nodes across 8 cores (5000 each) and routes
    every edge (its h row + dst index) to the core owning dst. A snake deal
    over degree-sorted nodes equalizes per-core edge counts; a proportional
    fill packs each core's nodes into 40 tiles of 128 nodes (desc degree
    order within the tile) under a chunk budget shared by all cores.
  - Segment-sum per 128-node tile runs on TensorE as matmuls over 128-edge
    chunks: PSUM[hid, n] += h_chunk.T @ onehot_chunk.
  - KEY TRICK: because the host places edges freely, most chunks use a
    CONSTANT one-hot ("fixed" chunks): pattern Bs maps slot e to node
    w*(128/s) + e//s, i.e. 128/s consecutive nodes x exactly s edge slots.
    With nodes degree-sorted, layered Bs chunks (s in 8,4,2,1) cover ~75%
    of edges with zero padding and ZERO DVE work (the one-hot is a
    preloaded constant). Only the remainder rides in "custom" chunks whose
    one-hot is built on DVE with iota + is_equal against the edge's dst
    slot (the expensive per-chunk op this trick minimizes).
  - The chunk plan (custom count + fixed layer structure) is equalized
    across cores (min layers / max customs) so one program serves all 8.
  - All tensors ride in bf16 (h, x, W, patterns, output); PSUM stays fp32.
    ScalarE applies relu + bias; the output leaves transposed and the host
    transposes it back.
"""

import os
from contextlib import ExitStack

import numpy as np
import ml_dtypes

import bass_rust
import concourse.bass as bass
import concourse.mybir as mybir
import concourse.tile as tile
from concourse.bass_utils import run_bass_kernel_spmd

N_NODES = 40000
N_EDGES = 640000
F = 128
HID = 128
N_CORES = 8
NPC = N_NODES // N_CORES  # nodes per core
P = 128
NT = (NPC + P - 1) // P  # node tiles per core
NPAD = NT * P  # padded nodes per core

BF16 = mybir.dt.bfloat16
NP_BF16 = ml_dtypes.bfloat16

# Fixed-pattern sizes (slots per node, window = 128/s nodes) and their
# column offsets inside the preloaded pattern tensor bpat [128, 240].
PAT_SIZES = (8, 4, 2, 1)
PAT_OFF = {8: 0, 4: 16, 2: 48, 1: 112}  # cumulative 16+32+64+128 = 240
PAT_COLS = 240

# Plan mode: "layered" (fixed+custom chunks) or "custom" (all-custom).
PLAN_MODE = os.environ.get("KERNEL_PLAN", "layered")
# Output-store granularity (tiles per output DMA) and h-DMA pairing.
OCHUNK = int(os.environ.get("KERNEL_OCHUNK", "10"))
HPAIR = int(os.environ.get("KERNEL_HPAIR", "2"))  # node tiles per h DMA


MAX_WAITS = 1
_split_cnt = [0]


def _split_excess_waits(nc, max_waits=MAX_WAITS):
    """This walrus build rejects more than MAX_WAITS sem waits on a single
    instruction. Hoist excess waits onto same-engine nops inserted just
    before the offending instruction."""
    for fn in nc.m.functions:
        for bb in fn.blocks:
            out = []
            changed = False
            for inst in bb.instructions:
                si = inst.sync_info
                waits = list(si.on_wait) if si is not None and si.on_wait else []
                if len(waits) > max_waits:
                    changed = True
                    head, keep = waits[:-max_waits], waits[-max_waits:]
                    for j in range(0, len(head), max_waits):
                        _split_cnt[0] += 1
                        nop = mybir.InstNoOp(
                            name=f"SWSPLIT-{_split_cnt[0]}", ins=[], outs=[]
                        )
                        nop.engine = inst.engine
                        nop.sync_info = bass_rust.SyncInfo(
                            on_wait=head[j : j + max_waits], on_update=[]
                        )
                        out.append(nop)
                    inst.sync_info = bass_rust.SyncInfo(
                        on_wait=keep, on_update=si.on_update
                    )
                out.append(inst)
            if changed:
                bb.instructions = out


def _route_nodes(deg):
    """Assign nodes to (core, slot) with per-core edge balance and tiles
    packed under a shared chunk budget, desc degree order within a tile."""
    order = np.argsort(-deg, kind="stable")
    snake = np.empty((NPC, N_CORES), dtype=np.int64)
    fwd = np.arange(N_CORES)
    for r in range(NPC):
        snake[r] = fwd if r % 2 == 0 else fwd[::-1]
    core_of = np.empty(N_NODES, dtype=np.int64)
    core_of[order] = snake.reshape(-1)

    ecore = np.zeros(N_CORES, dtype=np.int64)
    np.add.at(ecore, core_of, deg)

    node_slot = np.empty(N_NODES, dtype=np.int64)
    csum0 = int(-(-ecore.max() // P))
    for csum in range(csum0, csum0 + 16):
        base, rem = divmod(csum, NT)
        caps = np.full(NT, base, dtype=np.int64) * P
        caps[:rem] += P
        ok = True
        slots_all = []
        for c in range(N_CORES):
            nodes = np.where(core_of == c)[0]
            nd = deg[nodes]
            o = np.argsort(-nd, kind="stable")
            nodes, nd = nodes[o], nd[o]
            # proportional fill: each node (desc by degree) goes to the tile
            # with the largest remaining per-slot target deficit; assignment
            # order doubles as the within-tile rank (so ranks are desc-degree).
            tau = caps * (nd.sum() / caps.sum())
            load = np.zeros(NT, dtype=np.float64)
            nslots = np.full(NT, P, dtype=np.int64)
            tile_of = np.empty(NPC, dtype=np.int64)
            rank_of = np.empty(NPC, dtype=np.int64)
            for i in range(NPC):
                score = np.where(
                    nslots > 0, (tau - load) / np.maximum(nslots, 1), -1e18
                )
                t = int(np.argmax(score))
                tile_of[i] = t
                rank_of[i] = P - nslots[t]
                load[t] += nd[i]
                nslots[t] -= 1
            if (load.astype(np.int64) > caps).any():
                ok = False
                break
            slots_all.append((nodes, tile_of * P + rank_of))
        if ok:
            for c in range(N_CORES):
                nodes, slots = slots_all[c]
                node_slot[nodes] = slots
            return core_of, node_slot
    raise RuntimeError("binpack failed")


def _make_plan(degs):
    """degs: [N_CORES, NT, P] per-tile desc degrees. Returns per-tile plans:
    plan[t] = {"K": n_custom, "fixed": [(s, w), ...]} with layer structure
    equalized across cores (min layers, max customs)."""
    rem = degs.astype(np.int64).copy()
    plans = []
    for t in range(NT):
        fixed = []
        for s in PAT_SIZES:
            wn = P // s  # nodes per window
            nw = P // wn  # windows per tile
            seg = rem[:, t].reshape(N_CORES, nw, wn)
            L = (seg.min(axis=2) // s).min(axis=0)  # [nw] common layers
            for w in range(nw):
                fixed += [(s, w)] * int(L[w])
            rem[:, t] = (seg - (L[None, :, None] * s)).reshape(N_CORES, P)
        K = max(1, int(-(-rem[:, t].sum(axis=1).max() // P)))
        plans.append({"K": K, "fixed": fixed})
    return plans, rem


def preprocess(x, h, edge_dst, W, b):
    """Route/sort/pad inputs into one in_map per core. Returns
    (in_maps, plans, node_map)."""
    x = np.asarray(x, dtype=np.float32)
    h = np.asarray(h, dtype=np.float32)
    W = np.asarray(W, dtype=np.float32)
    b = np.asarray(b, dtype=np.float32)
    dst = np.asarray(edge_dst).astype(np.int64)

    deg = np.bincount(dst, minlength=N_NODES)
    node_core, node_slot = _route_nodes(deg)

    node_map = np.full((N_CORES, NPAD), -1, dtype=np.int64)
    node_map[node_core, node_slot] = np.arange(N_NODES, dtype=np.int64)

    degs = np.zeros((N_CORES, NT, P), dtype=np.int64)
    ids = node_map.reshape(N_CORES, NT, P)
    m = ids >= 0
    degs[m] = deg[ids[m]]

    if PLAN_MODE == "layered":
        plans, _rem = _make_plan(degs)
    else:
        plans = []
        for t in range(NT):
            K = int(-(-degs[:, t].sum(axis=1).max() // P))
            plans.append({"K": max(1, K), "fixed": []})

    Cs = [pl["K"] + len(pl["fixed"]) for pl in plans]
    Ctot = int(np.sum(Cs))
    ncust = int(np.sum([pl["K"] for pl in plans]))
    chunk0 = np.concatenate([[0], np.cumsum(Cs)])[:-1]  # first chunk of tile
    cust0 = np.concatenate([[0], np.cumsum([pl["K"] for pl in plans])])[:-1]

    # Per-core edge placement.
    h_s = np.zeros((N_CORES, P, Ctot, HID), dtype=NP_BF16)
    dstm = np.zeros((N_CORES, P, ncust), dtype=np.float32)
    for c in range(N_CORES):
        eidx = np.where(node_core[dst] == c)[0]
        key = node_slot[dst[eidx]]
        o = np.argsort(key, kind="stable")
        eidx, key = eidx[o], key[o]  # edges sorted by (tile, rank)
        starts = np.searchsorted(key, np.arange(NPAD))  # per-slot run start
        ends = np.searchsorted(key, np.arange(NPAD), side="right")
        used = np.zeros(NPAD, dtype=np.int64)  # consumed edges per node
        for t in range(NT):
            pl = plans[t]
            j = chunk0[t] + pl["K"]  # fixed chunks follow the customs
            for s, w in pl["fixed"]:
                wn = P // s
                nr = t * P + w * wn + np.arange(P) // s  # node of each slot
                eo = used[nr] + np.arange(P) % s
                sel = starts[nr] + eo
                h_s[c, :, j] = h[eidx[sel]].astype(NP_BF16)
                used[t * P + w * wn : t * P + (w + 1) * wn] += s
                j += 1
            # custom chunks: leftover edges, rank order
            nr = t * P + np.arange(P)
            cnt = ends[nr] - starts[nr] - used[nr]
            assert cnt.min() >= 0 and cnt.sum() <= pl["K"] * P
            rel = np.repeat(np.arange(P), cnt)  # dst slot per leftover edge
            if cnt.sum():
                sel = np.concatenate(
                    [
                        starts[nr[r]] + used[nr[r]] + np.arange(cnt[r])
                        for r in range(P)
                    ]
                )
            else:
                sel = np.empty(0, dtype=np.int64)
            for k in range(pl["K"]):
                j = chunk0[t] + k
                lo, hi = k * P, min((k + 1) * P, len(rel))
                n = max(0, hi - lo)
                if n > 0:
                    h_s[c, :n, j] = h[eidx[sel[lo:hi]]].astype(NP_BF16)
                    dstm[c, :n, cust0[t] + k] = rel[lo:hi]

    xT = np.zeros((N_CORES, F, NPAD), dtype=NP_BF16)
    for c in range(N_CORES):
        idsc = node_map[c]
        mm = idsc >= 0
        xT[c][:, mm] = x[idsc[mm]].astype(NP_BF16).T

    # constant one-hot patterns Bs[e, g] = (e//s == g), columns packed
    bpat = np.zeros((P, PAT_COLS), dtype=NP_BF16)
    for s in PAT_SIZES:
        e = np.arange(P)
        bpat[e, PAT_OFF[s] + e // s] = 1.0

    wT = np.ascontiguousarray(W.T.astype(NP_BF16))  # [F+HID, HID]
    bias = np.ascontiguousarray(b.reshape(HID, 1))

    in_maps = [
        {
            "hs": np.ascontiguousarray(h_s[c].reshape(P, Ctot * HID)),
            "dstm": np.ascontiguousarray(dstm[c]),
            "xT": np.ascontiguousarray(xT[c]),
            "bpat": bpat,
            "wT": wT,
            "bias": bias,
        }
        for c in range(N_CORES)
    ]
    return in_maps, plans, node_map


def build(
    plans,
    reps=1,
    loop_reps=1,
    do_hdma=True,
    do_cmp=True,
    do_mm=True,
    do_lin=True,
    do_act=True,
    do_out=True,
):
    Cs = [pl["K"] + len(pl["fixed"]) for pl in plans]
    Ctot = int(np.sum(Cs))
    ncust = int(np.sum([pl["K"] for pl in plans]))
    f32 = mybir.dt.float32

    nc = bass.Bass()
    hs = nc.dram_tensor("hs", [P, Ctot * HID], BF16, kind="ExternalInput")
    dstm = nc.dram_tensor("dstm", [P, ncust], f32, kind="ExternalInput")
    xT = nc.dram_tensor("xT", [F, NPAD], BF16, kind="ExternalInput")
    bpat_d = nc.dram_tensor("bpat", [P, PAT_COLS], BF16, kind="ExternalInput")
    wT = nc.dram_tensor("wT", [F + HID, HID], BF16, kind="ExternalInput")
    bias = nc.dram_tensor("bias", [HID, 1], f32, kind="ExternalInput")
    outT = nc.dram_tensor("outT", [HID, NPAD], BF16, kind="ExternalOutput")

    with tile.TileContext(nc) as tc, ExitStack() as ctx:
        const = ctx.enter_context(tc.tile_pool(name="const", bufs=1))
        inpool = ctx.enter_context(tc.tile_pool(name="inpool", bufs=1))
        hpool = ctx.enter_context(tc.tile_pool(name="hpool", bufs=3))
        ohpool = ctx.enter_context(tc.tile_pool(name="ohpool", bufs=8))
        hapool = ctx.enter_context(tc.tile_pool(name="hapool", bufs=2))
        opool = ctx.enter_context(tc.tile_pool(name="opool", bufs=1))
        psh = ctx.enter_context(tc.tile_pool(name="psh", bufs=4, space="PSUM"))
        pso = ctx.enter_context(tc.tile_pool(name="pso", bufs=4, space="PSUM"))

        iota_i = const.tile([P, P], mybir.dt.int32)
        nc.gpsimd.iota(iota_i[:], pattern=[[1, P]], base=0, channel_multiplier=0)
        iota_c = const.tile([P, P], BF16)
        nc.vector.tensor_copy(iota_c[:], iota_i[:])

        def body():
            # Per-execution input loads: small tensors ride the ACT HWDGE
            # ring; the big h stream has the SP ring to itself.
            dstm_sb = inpool.tile([P, ncust], f32, tag="dstm")
            nc.scalar.dma_start(dstm_sb[:], dstm[:])
            bp = inpool.tile([P, PAT_COLS], BF16, tag="bp")
            nc.scalar.dma_start(bp[:], bpat_d[:])
            wx = inpool.tile([P, HID], BF16, tag="wx")
            nc.scalar.dma_start(wx[:], wT[0:F, :])
            wh = inpool.tile([P, HID], BF16, tag="wh")
            nc.scalar.dma_start(wh[:], wT[F : F + HID, :])
            bt = inpool.tile([P, 1], f32, tag="bt")
            nc.scalar.dma_start(bt[:], bias[:])
            xall = inpool.tile([P, NPAD], BF16, tag="xall")
            nc.scalar.dma_start(xall[:], xT[:])
            obuf = None
            if (do_act and do_lin) or do_out:
                obuf = opool.tile([P, NPAD], BF16, tag="obuf", name="obuf")

            # h tiles come HPAIR node-tiles per DMA (~1MB) for bandwidth.
            h_group = {}

            def load_group(t0):
                n = sum(Cs[t0 : t0 + HPAIR])
                j0 = sum(Cs[:t0])
                ht = hpool.tile([P, n * HID], BF16, tag="h_t", name="ht")
                if do_hdma:
                    nc.sync.dma_start(ht[:], hs[:, j0 * HID : (j0 + n) * HID])
                elif do_mm:
                    nc.gpsimd.memset(ht[:, 0:1], 0.0)  # probe: mark written
                for tt in range(t0, min(t0 + HPAIR, NT)):
                    off = sum(Cs[t0:tt])
                    h_group[tt] = (ht, off)

            ci = 0  # global custom-chunk index
            for t in range(NT):
                pl = plans[t]
                K, fixed = pl["K"], pl["fixed"]
                if t % HPAIR == 0:
                    load_group(t)
                ht, off = h_group.pop(t)

                def hsl(k, ht=ht, off=off):
                    return ht[:, (off + k) * HID : (off + k + 1) * HID]

                oh = None
                if do_cmp or do_mm:
                    oh = ohpool.tile([P, K * P], BF16, tag="oh", name="oh")
                if do_mm and not do_cmp:
                    nc.gpsimd.memset(oh[:, 0:1], 0.0)  # probe: mark written
                if do_cmp:
                    for k in range(K):
                        # custom one-hot: row e = (iota == dst[e]); bf16
                        # unit-stride operands keep DVE in its fast mode.
                        nc.vector.tensor_scalar(
                            out=oh[:, k * P : (k + 1) * P],
                            in0=iota_c[:],
                            scalar1=dstm_sb[:, ci + k : ci + k + 1],
                            scalar2=None,
                            op0=mybir.AluOpType.is_equal,
                        )
                ph = None
                if do_mm:
                    ph = psh.tile([P, P], f32, tag="ph", name="ph")
                    nmm = K + len(fixed)
                    # custom chunks first: chunk 0 start=True clears the bank
                    # and writes the full 128-column width.
                    for k in range(K):
                        nc.tensor.matmul(
                            out=ph[:],
                            lhsT=hsl(k),
                            rhs=oh[:, k * P : (k + 1) * P],
                            start=(k == 0),
                            stop=(k == nmm - 1),
                        )
                    for i, (s, w) in enumerate(fixed):
                        wn = P // s
                        nc.tensor.matmul(
                            out=ph[:, w * wn : (w + 1) * wn],
                            lhsT=hsl(K + i),
                            rhs=bp[:, PAT_OFF[s] : PAT_OFF[s] + wn],
                            start=False,
                            stop=(K + i == nmm - 1),
                        )
                ci += K
                hA = None
                if do_act and do_mm:
                    hA = hapool.tile([P, P], BF16, tag="hA", name="hA")
                    nc.scalar.copy(hA[:], ph[:])
                po = None
                if do_lin:
                    po = pso.tile([P, P], f32, tag="po", name="po")
                    nc.tensor.matmul(
                        out=po[:],
                        lhsT=wx[:],
                        rhs=xall[:, t * P : (t + 1) * P],
                        start=True,
                        stop=False,
                    )
                    rhs2 = hA if (do_act and do_mm) else xall[:, t * P : (t + 1) * P]
                    nc.tensor.matmul(
                        out=po[:], lhsT=wh[:], rhs=rhs2, start=False, stop=True
                    )
                if do_act and do_lin:
                    nc.scalar.activation(
                        obuf[:, t * P : (t + 1) * P],
                        po[:],
                        mybir.ActivationFunctionType.Relu,
                        bias=bt[:, :1],
                    )
                elif do_out:
                    nc.scalar.copy(
                        obuf[:, t * P : (t + 1) * P], xall[:, t * P : (t + 1) * P]
                    )
                if do_out and ((t + 1) % OCHUNK == 0 or t == NT - 1):
                    lo = (t // OCHUNK) * OCHUNK
                    nc.scalar.dma_start(
                        outT[:, lo * P : (t + 1) * P], obuf[:, lo * P : (t + 1) * P]
                    )

        for _rep in range(reps):
            if loop_reps > 1:
                with tc.For_i(0, loop_reps):
                    body()
            else:
                body()
    return nc


def postprocess(results, node_map):
    out = np.empty((N_NODES, HID), dtype=np.float32)
    for c in range(N_CORES):
        ids = node_map[c]
        mask = ids >= 0
        out[ids[mask]] = results[c]["outT"].astype(np.float32).T[mask]
    return out


def kernel(x, h, edge_dst, W, b, **_kw):
    in_maps, plans, node_map = preprocess(x, h, edge_dst, W, b)
    nc = build(plans)
    _split_excess_waits(nc)  # HW-only pass (the sim race detector rejects it)
    results = None
    last_err = None
    for _attempt in range(3):  # device occasionally reports a transient
        try:  # NRT_EXEC_UNIT_UNRECOVERABLE right after a heavy prior session
            res = run_bass_kernel_spmd(nc, in_maps, list(range(N_CORES)))
            results = res.results
            break
        except ModuleNotFoundError:
            # trace path needs antenv.axon_hooks, absent in trimmed clients
            from concourse import bass2jax

            results = bass2jax.run_bass_via_pjrt(nc, in_maps, n_cores=N_CORES)
            break
        except Exception as e:  # noqa: BLE001
            last_err = e
            if "UNRECOVERABLE" not in str(e) and "UNAVAILABLE" not in str(e):
                raise
            import time as _time

            _time.sleep(10)
    if results is None:
        raise last_err
    return postprocess(results, node_map)


# revision 10
# speedup vs baseline: 1.7539x; 1.0043x over previous
"""DMPNN last layer on 8 Trainium2 NeuronCores.

out = relu(concat([x, segment_sum(h, edge_dst, N)], 1) @ W.T + b)

Strategy (graph-parallel, no collectives):
  - Host partitions the 40000 nodes across 8 cores (5000 each) and routes
    every edge (its h row + dst index) to the core owning dst. A snake deal
    over degree-sorted nodes equalizes per-core edge counts; a proportional
    fill packs each core's nodes into 40 tiles of 128 nodes (desc degree
    order within the tile) under a chunk budget shared by all cores.
  - Segment-sum per 128-node tile runs on TensorE as matmuls over 128-edge
    chunks: PSUM[hid, n] += h_chunk.T @ onehot_chunk.
  - KEY TRICK: because the host places edges freely, most chunks use a
    CONSTANT one-hot ("fixed" chunks): pattern Bs maps slot e to node
    w*(128/s) + e//s, i.e. 128/s consecutive nodes x exactly s edge slots.
    With nodes degree-sorted, layered Bs chunks (s in 8,4,2,1) cover ~75%
    of edges with zero padding and ZERO DVE work (the one-hot is a
    preloaded constant). Only the remainder rides in "custom" chunks whose
    one-hot is built on DVE with iota + is_equal against the edge's dst
    slot (the expensive per-chunk op this trick minimizes).
  - The chunk plan (custom count + fixed layer structure) is equalized
    across cores (min layers / max customs) so one program serves all 8.
  - All tensors ride in bf16 (h, x, W, patterns, output); PSUM stays fp32.
    ScalarE applies relu + bias; the output leaves transposed and the host
    transposes it back.
"""

import os
from contextlib import ExitStack

import numpy as np
import ml_dtypes

import bass_rust
import concourse.bass as bass
import concourse.mybir as mybir
import concourse.tile as tile
from concourse.bass_utils import run_bass_kernel_spmd

N_NODES = 40000
N_EDGES = 640000
F = 128
HID = 128
N_CORES = 8
NPC = N_NODES // N_CORES  # nodes per core
P = 128
NT = (NPC + P - 1) // P  # node tiles per core
NPAD = NT * P  # padded nodes per core

BF16 = mybir.dt.bfloat16
NP_BF16 = ml_dtypes.bfloat16

# Fixed-pattern sizes (slots per node, window = 128/s nodes) and their
# column offsets inside the preloaded pattern tensor bpat [128, 240].
PAT_SIZES = (8, 4, 2, 1)
PAT_OFF = {8: 0, 4: 16, 2: 48, 1: 112}  # cumulative 16+32+64+128 = 240
PAT_COLS = 240

# Plan mode: "layered" (fixed+custom chunks) or "custom" (all-custom).
PLAN_MODE = os.environ.get("KERNEL_PLAN", "layered")
# Output-store granularity (tiles per output DMA) and h-DMA pairing.
OCHUNK = int(os.environ.get("KERNEL_OCHUNK", "10"))
HPAIR = int(os.environ.get("KERNEL_HPAIR", "2"))  # node tiles per h DMA


MAX_WAITS = 1
_split_cnt = [0]


def _split_excess_waits(nc, max_waits=MAX_WAITS):
    """This walrus build rejects more than MAX_WAITS sem waits on a single
    instruction. Hoist excess waits onto same-engine nops inserted just
    before the offending instruction."""
    for fn in nc.m.functions:
        for bb in fn.blocks:
            out = []
            changed = False
            for inst in bb.instructions:
                si = inst.sync_info
                waits = list(si.on_wait) if si is not None and si.on_wait else []
                if len(waits) > max_waits:
                    changed = True
                    head, keep = waits[:-max_waits], waits[-max_waits:]
                    for j in range(0, len(head), max_waits):
                        _split_cnt[0] += 1
                        nop = mybir.InstNoOp(
                            name=f"SWSPLIT-{_split_cnt[0]}", ins=[], outs=[]
                        )
                        nop.engine = inst.engine
                        nop.sync_info = bass_rust.SyncInfo(
                            on_wait=head[j : j + max_waits], on_update=[]
                        )
                        out.append(nop)
                    inst.sync_info = bass_rust.SyncInfo(
                        on_wait=keep, on_update=si.on_update
                    )
                out.append(inst)
            if changed:
                bb.instructions = out


def _route_nodes(deg):
    """Assign nodes to (core, slot) with per-core edge balance and tiles
    packed under a shared chunk budget, desc degree order within a tile."""
    order = np.argsort(-deg, kind="stable")
    snake = np.empty((NPC, N_CORES), dtype=np.int64)
    fwd = np.arange(N_CORES)
    for r in range(NPC):
        snake[r] = fwd if r % 2 == 0 else fwd[::-1]
    core_of = np.empty(N_NODES, dtype=np.int64)
    core_of[order] = snake.reshape(-1)

    ecore = np.zeros(N_CORES, dtype=np.int64)
    np.add.at(ecore, core_of, deg)

    node_slot = np.empty(N_NODES, dtype=np.int64)
    csum0 = int(-(-ecore.max() // P))
    for csum in range(csum0, csum0 + 16):
        base, rem = divmod(csum, NT)
        caps = np.full(NT, base, dtype=np.int64) * P
        caps[:rem] += P
        ok = True
        slots_all = []
        for c in range(N_CORES):
            nodes = np.where(core_of == c)[0]
            nd = deg[nodes]
            o = np.argsort(-nd, kind="stable")
            nodes, nd = nodes[o], nd[o]
            # proportional fill: each node (desc by degree) goes to the tile
            # with the largest remaining per-slot target deficit; assignment
            # order doubles as the within-tile rank (so ranks are desc-degree).
            tau = caps * (nd.sum() / caps.sum())
            load = np.zeros(NT, dtype=np.float64)
            nslots = np.full(NT, P, dtype=np.int64)
            tile_of = np.empty(NPC, dtype=np.int64)
            rank_of = np.empty(NPC, dtype=np.int64)
            for i in range(NPC):
                score = np.where(
                    nslots > 0, (tau - load) / np.maximum(nslots, 1), -1e18
                )
                t = int(np.argmax(score))
                tile_of[i] = t
                rank_of[i] = P - nslots[t]
                load[t] += nd[i]
                nslots[t] -= 1
            if (load.astype(np.int64) > caps).any():
                ok = False
                break
            slots_all.append((nodes, tile_of * P + rank_of))
        if ok:
            for c in range(N_CORES):
                nodes, slots = slots_all[c]
                node_slot[nodes] = slots
            return core_of, node_slot
    raise RuntimeError("binpack failed")


def _make_plan(degs):
    """degs: [N_CORES, NT, P] per-tile desc degrees. Returns per-tile plans:
    plan[t] = {"K": n_custom, "fixed": [(s, w), ...]} with layer structure
    equalized across cores (min layers, max customs)."""
    rem = degs.astype(np.int64).copy()
    plans = []
    for t in range(NT):
        fixed = []
        for s in PAT_SIZES:
            wn = P // s  # nodes per window
            nw = P // wn  # windows per tile
            seg = rem[:, t].reshape(N_CORES, nw, wn)
            L = (seg.min(axis=2) // s).min(axis=0)  # [nw] common layers
            for w in range(nw):
                fixed += [(s, w)] * int(L[w])
            rem[:, t] = (seg - (L[None, :, None] * s)).reshape(N_CORES, P)
        K = max(1, int(-(-rem[:, t].sum(axis=1).max() // P)))
        plans.append({"K": K, "fixed": fixed})
    return plans, rem


def preprocess(x, h, edge_dst, W, b):
    """Route/sort/pad inputs into one in_map per core. Returns
    (in_maps, plans, node_map)."""
    x = np.asarray(x, dtype=np.float32)
    h = np.asarray(h, dtype=np.float32)
    W = np.asarray(W, dtype=np.float32)
    b = np.asarray(b, dtype=np.float32)
    dst = np.asarray(edge_dst).astype(np.int64)

    deg = np.bincount(dst, minlength=N_NODES)
    node_core, node_slot = _route_nodes(deg)

    node_map = np.full((N_CORES, NPAD), -1, dtype=np.int64)
    node_map[node_core, node_slot] = np.arange(N_NODES, dtype=np.int64)

    degs = np.zeros((N_CORES, NT, P), dtype=np.int64)
    ids = node_map.reshape(N_CORES, NT, P)
    m = ids >= 0
    degs[m] = deg[ids[m]]

    if PLAN_MODE == "layered":
        plans, _rem = _make_plan(degs)
    else:
        plans = []
        for t in range(NT):
            K = int(-(-degs[:, t].sum(axis=1).max() // P))
            plans.append({"K": max(1, K), "fixed": []})

    Cs = [pl["K"] + len(pl["fixed"]) for pl in plans]
    Ctot = int(np.sum(Cs))
    ncust = int(np.sum([pl["K"] for pl in plans]))
    chunk0 = np.concatenate([[0], np.cumsum(Cs)])[:-1]  # first chunk of tile
    cust0 = np.concatenate([[0], np.cumsum([pl["K"] for pl in plans])])[:-1]

    # Per-core edge placement.
    h_s = np.zeros((N_CORES, P, Ctot, HID), dtype=NP_BF16)
    dstm = np.zeros((N_CORES, P, ncust), dtype=np.float32)
    for c in range(N_CORES):
        eidx = np.where(node_core[dst] == c)[0]
        key = node_slot[dst[eidx]]
        o = np.argsort(key, kind="stable")
        eidx, key = eidx[o], key[o]  # edges sorted by (tile, rank)
        starts = np.searchsorted(key, np.arange(NPAD))  # per-slot run start
        ends = np.searchsorted(key, np.arange(NPAD), side="right")
        used = np.zeros(NPAD, dtype=np.int64)  # consumed edges per node
        for t in range(NT):
            pl = plans[t]
            j = chunk0[t] + pl["K"]  # fixed chunks follow the customs
            for s, w in pl["fixed"]:
                wn = P // s
                nr = t * P + w * wn + np.arange(P) // s  # node of each slot
                eo = used[nr] + np.arange(P) % s
                sel = starts[nr] + eo
                h_s[c, :, j] = h[eidx[sel]].astype(NP_BF16)
                used[t * P + w * wn : t * P + (w + 1) * wn] += s
                j += 1
            # custom chunks: leftover edges, rank order
            nr = t * P + np.arange(P)
            cnt = ends[nr] - starts[nr] - used[nr]
            assert cnt.min() >= 0 and cnt.sum() <= pl["K"] * P
            rel = np.repeat(np.arange(P), cnt)  # dst slot per leftover edge
            if cnt.sum():
                sel = np.concatenate(
                    [
                        starts[nr[r]] + used[nr[r]] + np.arange(cnt[r])
                        for r in range(P)
                    ]
                )
            else:
                sel = np.empty(0, dtype=np.int64)
            for k in range(pl["K"]):
                j = chunk0[t] + k
                lo, hi = k * P, min((k + 1) * P, len(rel))
                n = max(0, hi - lo)
                if n > 0:
                    h_s[c, :n, j] = h[eidx[sel[lo:hi]]].astype(NP_BF16)
                    dstm[c, :n, cust0[t] + k] = rel[lo:hi]

    xT = np.zeros((N_CORES, F, NPAD), dtype=NP_BF16)
    for c in range(N_CORES):
        idsc = node_map[c]
        mm = idsc >= 0
        xT[c][:, mm] = x[idsc[mm]].astype(NP_BF16).T

    # constant one-hot patterns Bs[e, g] = (e//s == g), columns packed
    bpat = np.zeros((P, PAT_COLS), dtype=NP_BF16)
    for s in PAT_SIZES:
        e = np.arange(P)
        bpat[e, PAT_OFF[s] + e // s] = 1.0

    wT = np.ascontiguousarray(W.T.astype(NP_BF16))  # [F+HID, HID]
    bias = np.ascontiguousarray(b.reshape(HID, 1))

    in_maps = [
        {
            "hs": np.ascontiguousarray(h_s[c].reshape(P, Ctot * HID)),
            "dstm": np.ascontiguousarray(dstm[c]),
            "xT": np.ascontiguousarray(xT[c]),
            "bpat": bpat,
            "wT": wT,
            "bias": bias,
        }
        for c in range(N_CORES)
    ]
    return in_maps, plans, node_map


def build(
    plans,
    reps=1,
    loop_reps=1,
    do_hdma=True,
    do_cmp=True,
    do_mm=True,
    do_lin=True,
    do_act=True,
    do_out=True,
):
    Cs = [pl["K"] + len(pl["fixed"]) for pl in plans]
    Ctot = int(np.sum(Cs))
    ncust = int(np.sum([pl["K"] for pl in plans]))
    f32 = mybir.dt.float32

    nc = bass.Bass()
    hs = nc.dram_tensor("hs", [P, Ctot * HID], BF16, kind="ExternalInput")
    dstm = nc.dram_tensor("dstm", [P, ncust], f32, kind="ExternalInput")
    xT = nc.dram_tensor("xT", [F, NPAD], BF16, kind="ExternalInput")
    bpat_d = nc.dram_tensor("bpat", [P, PAT_COLS], BF16, kind="ExternalInput")
    wT = nc.dram_tensor("wT", [F + HID, HID], BF16, kind="ExternalInput")
    bias = nc.dram_tensor("bias", [HID, 1], f32, kind="ExternalInput")
    outT = nc.dram_tensor("outT", [HID, NPAD], BF16, kind="ExternalOutput")

    with tile.TileContext(nc) as tc, ExitStack() as ctx:
        const = ctx.enter_context(tc.tile_pool(name="const", bufs=1))
        inpool = ctx.enter_context(tc.tile_pool(name="inpool", bufs=2))
        hpool = ctx.enter_context(tc.tile_pool(name="hpool", bufs=4))
        ohpool = ctx.enter_context(tc.tile_pool(name="ohpool", bufs=8))
        hapool = ctx.enter_context(tc.tile_pool(name="hapool", bufs=2))
        opool = ctx.enter_context(tc.tile_pool(name="opool", bufs=2))
        psh = ctx.enter_context(tc.tile_pool(name="psh", bufs=4, space="PSUM"))
        pso = ctx.enter_context(tc.tile_pool(name="pso", bufs=4, space="PSUM"))

        iota_i = const.tile([P, P], mybir.dt.int32)
        nc.gpsimd.iota(iota_i[:], pattern=[[1, P]], base=0, channel_multiplier=0)
        iota_c = const.tile([P, P], BF16)
        nc.vector.tensor_copy(iota_c[:], iota_i[:])

        def body():
            # Per-execution input loads: small tensors ride the ACT HWDGE
            # ring; the big h stream has the SP ring to itself.
            dstm_sb = inpool.tile([P, ncust], f32, tag="dstm")
            nc.scalar.dma_start(dstm_sb[:], dstm[:])
            bp = inpool.tile([P, PAT_COLS], BF16, tag="bp")
            nc.scalar.dma_start(bp[:], bpat_d[:])
            wx = inpool.tile([P, HID], BF16, tag="wx")
            nc.scalar.dma_start(wx[:], wT[0:F, :])
            wh = inpool.tile([P, HID], BF16, tag="wh")
            nc.scalar.dma_start(wh[:], wT[F : F + HID, :])
            bt = inpool.tile([P, 1], f32, tag="bt")
            nc.scalar.dma_start(bt[:], bias[:])
            xall = inpool.tile([P, NPAD], BF16, tag="xall")
            nc.scalar.dma_start(xall[:], xT[:])
            obuf = None
            if (do_act and do_lin) or do_out:
                obuf = opool.tile([P, NPAD], BF16, tag="obuf", name="obuf")

            # h tiles come HPAIR node-tiles per DMA (~1MB) for bandwidth.
            h_group = {}

            def load_group(t0):
                n = sum(Cs[t0 : t0 + HPAIR])
                j0 = sum(Cs[:t0])
                ht = hpool.tile([P, n * HID], BF16, tag="h_t", name="ht")
                if do_hdma:
                    nc.sync.dma_start(ht[:], hs[:, j0 * HID : (j0 + n) * HID])
                elif do_mm:
                    nc.gpsimd.memset(ht[:, 0:1], 0.0)  # probe: mark written
                for tt in range(t0, min(t0 + HPAIR, NT)):
                    off = sum(Cs[t0:tt])
                    h_group[tt] = (ht, off)

            ci = 0  # global custom-chunk index
            for t in range(NT):
                pl = plans[t]
                K, fixed = pl["K"], pl["fixed"]
                if t % HPAIR == 0:
                    load_group(t)
                ht, off = h_group.pop(t)

                def hsl(k, ht=ht, off=off):
                    return ht[:, (off + k) * HID : (off + k + 1) * HID]

                oh = None
                if do_cmp or do_mm:
                    oh = ohpool.tile([P, K * P], BF16, tag="oh", name="oh")
                if do_mm and not do_cmp:
                    nc.gpsimd.memset(oh[:, 0:1], 0.0)  # probe: mark written
                if do_cmp:
                    for k in range(K):
                        # custom one-hot: row e = (iota == dst[e]); bf16
                        # unit-stride operands keep DVE in its fast mode.
                        nc.vector.tensor_scalar(
                            out=oh[:, k * P : (k + 1) * P],
                            in0=iota_c[:],
                            scalar1=dstm_sb[:, ci + k : ci + k + 1],
                            scalar2=None,
                            op0=mybir.AluOpType.is_equal,
                        )
                ph = None
                if do_mm:
                    ph = psh.tile([P, P], f32, tag="ph", name="ph")
                    nmm = K + len(fixed)
                    # custom chunks first: chunk 0 start=True clears the bank
                    # and writes the full 128-column width.
                    for k in range(K):
                        nc.tensor.matmul(
                            out=ph[:],
                            lhsT=hsl(k),
                            rhs=oh[:, k * P : (k + 1) * P],
                            start=(k == 0),
                            stop=(k == nmm - 1),
                        )
                    for i, (s, w) in enumerate(fixed):
                        wn = P // s
                        nc.tensor.matmul(
                            out=ph[:, w * wn : (w + 1) * wn],
                            lhsT=hsl(K + i),
                            rhs=bp[:, PAT_OFF[s] : PAT_OFF[s] + wn],
                            start=False,
                            stop=(K + i == nmm - 1),
                        )
                ci += K
                hA = None
                if do_act and do_mm:
                    hA = hapool.tile([P, P], BF16, tag="hA", name="hA")
                    nc.scalar.copy(hA[:], ph[:])
                po = None
                if do_lin:
                    po = pso.tile([P, P], f32, tag="po", name="po")
                    nc.tensor.matmul(
                        out=po[:],
                        lhsT=wx[:],
                        rhs=xall[:, t * P : (t + 1) * P],
                        start=True,
                        stop=False,
                    )
                    rhs2 = hA if (do_act and do_mm) else xall[:, t * P : (t + 1) * P]
                    nc.tensor.matmul(
                        out=po[:], lhsT=wh[:], rhs=rhs2, start=False, stop=True
                    )
                if do_act and do_lin:
                    nc.scalar.activation(
                        obuf[:, t * P : (t + 1) * P],
                        po[:],
                        mybir.ActivationFunctionType.Relu,
                        bias=bt[:, :1],
                    )
                elif do_out:
                    nc.scalar.copy(
                        obuf[:, t * P : (t + 1) * P], xall[:, t * P : (t + 1) * P]
                    )
                if do_out and ((t + 1) % OCHUNK == 0 or t == NT - 1):
                    lo = (t // OCHUNK) * OCHUNK
                    nc.scalar.dma_start(
                        outT[:, lo * P : (t + 1) * P], obuf[:, lo * P : (t + 1) * P]
                    )

        for _rep in range(reps):
            if loop_reps > 1:
                with tc.For_i(0, loop_reps):
                    body()
            else:
                body()
    return nc


def postprocess(results, node_map):
    out = np.empty((N_NODES, HID), dtype=np.float32)
    for c in range(N_CORES):
        ids = node_map[c]
        mask = ids >= 0
        out[ids[mask]] = results[c]["outT"].astype(np.float32).T[mask]
    return out


def kernel(x, h, edge_dst, W, b, **_kw):
    in_maps, plans, node_map = preprocess(x, h, edge_dst, W, b)
    nc = build(plans)
    _split_excess_waits(nc)  # HW-only pass (the sim race detector rejects it)
    results = None
    last_err = None
    for _attempt in range(3):  # device occasionally reports a transient
        try:  # NRT_EXEC_UNIT_UNRECOVERABLE right after a heavy prior session
            res = run_bass_kernel_spmd(nc, in_maps, list(range(N_CORES)))
            results = res.results
            break
        except ModuleNotFoundError:
            # trace path needs antenv.axon_hooks, absent in trimmed clients
            from concourse import bass2jax

            results = bass2jax.run_bass_via_pjrt(nc, in_maps, n_cores=N_CORES)
            break
        except Exception as e:  # noqa: BLE001
            last_err = e
            if "UNRECOVERABLE" not in str(e) and "UNAVAILABLE" not in str(e):
                raise
            import time as _time

            _time.sleep(10)
    if results is None:
        raise last_err
    return postprocess(results, node_map)


# revision 12
# speedup vs baseline: 2.2101x; 1.2601x over previous
"""DMPNN last layer on 8 Trainium2 NeuronCores.

out = relu(concat([x, segment_sum(h, edge_dst, N)], 1) @ W.T + b)

Strategy (graph-parallel, no collectives):
  - Host partitions the 40000 nodes across 8 cores (5000 each) and routes
    every edge (its h row + dst index) to the core owning dst. A snake deal
    over degree-sorted nodes equalizes per-core edge counts; a proportional
    fill packs each core's nodes into 40 tiles of 128 nodes (desc degree
    order within the tile) under a chunk budget shared by all cores.
  - Segment-sum per 128-node tile runs on TensorE as matmuls over 128-edge
    chunks: PSUM[hid, n] += h_chunk.T @ onehot_chunk.
  - KEY TRICK: because the host places edges freely, most chunks use a
    CONSTANT one-hot ("fixed" chunks): pattern Bs maps slot e to node
    w*(128/s) + e//s, i.e. 128/s consecutive nodes x exactly s edge slots.
    With nodes degree-sorted, layered Bs chunks (s in 8,4,2,1) cover ~75%
    of edges with zero padding and ZERO DVE work (the one-hot is a
    preloaded constant). Only the remainder rides in "custom" chunks whose
    one-hot is built on DVE with iota + is_equal against the edge's dst
    slot (the expensive per-chunk op this trick minimizes).
  - The chunk plan (custom count + fixed layer structure) is equalized
    across cores (min layers / max customs) so one program serves all 8.
  - All tensors ride in bf16 (h, x, W, patterns, output); PSUM stays fp32.
    ScalarE applies relu + bias; the output leaves transposed and the host
    transposes it back.
"""

import os
from contextlib import ExitStack

import numpy as np
import ml_dtypes

import bass_rust
import concourse.bass as bass
import concourse.mybir as mybir
import concourse.tile as tile
from concourse.bass_utils import run_bass_kernel_spmd

N_NODES = 40000
N_EDGES = 640000
F = 128
HID = 128
N_CORES = 8
NPC = N_NODES // N_CORES  # nodes per core
P = 128
NT = (NPC + P - 1) // P  # node tiles per core
NPAD = NT * P  # padded nodes per core

BF16 = mybir.dt.bfloat16
NP_BF16 = ml_dtypes.bfloat16

# Fixed-pattern sizes (slots per node, window = 128/s nodes) and their
# column offsets inside the preloaded pattern tensor bpat [128, 240].
PAT_SIZES = (8, 4, 2, 1)
PAT_OFF = {8: 0, 4: 16, 2: 48, 1: 112}  # cumulative 16+32+64+128 = 240
PAT_COLS = 240

# Plan mode: "layered" (fixed+custom chunks) or "custom" (all-custom).
PLAN_MODE = os.environ.get("KERNEL_PLAN", "layered")
# Output-store granularity (tiles per output DMA) and h-DMA pairing.
OCHUNK = int(os.environ.get("KERNEL_OCHUNK", "10"))
HPAIR = int(os.environ.get("KERNEL_HPAIR", "2"))  # node tiles per h DMA


MAX_WAITS = 1
_split_cnt = [0]


def _split_excess_waits(nc, max_waits=MAX_WAITS):
    """This walrus build rejects more than MAX_WAITS sem waits on a single
    instruction. Hoist excess waits onto same-engine nops inserted just
    before the offending instruction."""
    for fn in nc.m.functions:
        for bb in fn.blocks:
            out = []
            changed = False
            for inst in bb.instructions:
                si = inst.sync_info
                waits = list(si.on_wait) if si is not None and si.on_wait else []
                if len(waits) > max_waits:
                    changed = True
                    head, keep = waits[:-max_waits], waits[-max_waits:]
                    for j in range(0, len(head), max_waits):
                        _split_cnt[0] += 1
                        nop = mybir.InstNoOp(
                            name=f"SWSPLIT-{_split_cnt[0]}", ins=[], outs=[]
                        )
                        nop.engine = inst.engine
                        nop.sync_info = bass_rust.SyncInfo(
                            on_wait=head[j : j + max_waits], on_update=[]
                        )
                        out.append(nop)
                    inst.sync_info = bass_rust.SyncInfo(
                        on_wait=keep, on_update=si.on_update
                    )
                out.append(inst)
            if changed:
                bb.instructions = out


def _route_nodes(deg):
    """Assign nodes to (core, slot) with per-core edge balance and tiles
    packed under a shared chunk budget, desc degree order within a tile."""
    order = np.argsort(-deg, kind="stable")
    snake = np.empty((NPC, N_CORES), dtype=np.int64)
    fwd = np.arange(N_CORES)
    for r in range(NPC):
        snake[r] = fwd if r % 2 == 0 else fwd[::-1]
    core_of = np.empty(N_NODES, dtype=np.int64)
    core_of[order] = snake.reshape(-1)

    ecore = np.zeros(N_CORES, dtype=np.int64)
    np.add.at(ecore, core_of, deg)

    node_slot = np.empty(N_NODES, dtype=np.int64)
    csum0 = int(-(-ecore.max() // P))
    for csum in range(csum0, csum0 + 16):
        base, rem = divmod(csum, NT)
        caps = np.full(NT, base, dtype=np.int64) * P
        caps[:rem] += P
        ok = True
        slots_all = []
        for c in range(N_CORES):
            nodes = np.where(core_of == c)[0]
            nd = deg[nodes]
            o = np.argsort(-nd, kind="stable")
            nodes, nd = nodes[o], nd[o]
            # proportional fill: each node (desc by degree) goes to the tile
            # with the largest remaining per-slot target deficit; assignment
            # order doubles as the within-tile rank (so ranks are desc-degree).
            tau = caps * (nd.sum() / caps.sum())
            load = np.zeros(NT, dtype=np.float64)
            nslots = np.full(NT, P, dtype=np.int64)
            tile_of = np.empty(NPC, dtype=np.int64)
            rank_of = np.empty(NPC, dtype=np.int64)
            for i in range(NPC):
                score = np.where(
                    nslots > 0, (tau - load) / np.maximum(nslots, 1), -1e18
                )
                t = int(np.argmax(score))
                tile_of[i] = t
                rank_of[i] = P - nslots[t]
                load[t] += nd[i]
                nslots[t] -= 1
            if (load.astype(np.int64) > caps).any():
                ok = False
                break
            slots_all.append((nodes, tile_of * P + rank_of))
        if ok:
            for c in range(N_CORES):
                nodes, slots = slots_all[c]
                node_slot[nodes] = slots
            return core_of, node_slot
    raise RuntimeError("binpack failed")


def _make_plan(degs):
    """degs: [N_CORES, NT, P] per-tile desc degrees. Returns per-tile plans:
    plan[t] = {"K": n_custom, "fixed": [(s, w), ...]} with layer structure
    equalized across cores (min layers, max customs)."""
    rem = degs.astype(np.int64).copy()
    plans = []
    for t in range(NT):
        fixed = []
        for s in PAT_SIZES:
            wn = P // s  # nodes per window
            nw = P // wn  # windows per tile
            seg = rem[:, t].reshape(N_CORES, nw, wn)
            L = (seg.min(axis=2) // s).min(axis=0)  # [nw] common layers
            for w in range(nw):
                fixed += [(s, w)] * int(L[w])
            rem[:, t] = (seg - (L[None, :, None] * s)).reshape(N_CORES, P)
        K = max(1, int(-(-rem[:, t].sum(axis=1).max() // P)))
        plans.append({"K": K, "fixed": fixed})
    return plans, rem


def preprocess(x, h, edge_dst, W, b):
    """Route/sort/pad inputs into one in_map per core. Returns
    (in_maps, plans, node_map)."""
    x = np.asarray(x, dtype=np.float32)
    h = np.asarray(h, dtype=np.float32)
    W = np.asarray(W, dtype=np.float32)
    b = np.asarray(b, dtype=np.float32)
    dst = np.asarray(edge_dst).astype(np.int64)

    deg = np.bincount(dst, minlength=N_NODES)
    node_core, node_slot = _route_nodes(deg)

    node_map = np.full((N_CORES, NPAD), -1, dtype=np.int64)
    node_map[node_core, node_slot] = np.arange(N_NODES, dtype=np.int64)

    degs = np.zeros((N_CORES, NT, P), dtype=np.int64)
    ids = node_map.reshape(N_CORES, NT, P)
    m = ids >= 0
    degs[m] = deg[ids[m]]

    if PLAN_MODE == "layered":
        plans, _rem = _make_plan(degs)
    else:
        plans = []
        for t in range(NT):
            K = int(-(-degs[:, t].sum(axis=1).max() // P))
            plans.append({"K": max(1, K), "fixed": []})

    Cs = [pl["K"] + len(pl["fixed"]) for pl in plans]
    Ctot = int(np.sum(Cs))
    ncust = int(np.sum([pl["K"] for pl in plans]))
    chunk0 = np.concatenate([[0], np.cumsum(Cs)])[:-1]  # first chunk of tile
    cust0 = np.concatenate([[0], np.cumsum([pl["K"] for pl in plans])])[:-1]

    # Per-core edge placement.
    h_s = np.zeros((N_CORES, P, Ctot, HID), dtype=NP_BF16)
    dstm = np.zeros((N_CORES, P, ncust), dtype=np.float32)
    for c in range(N_CORES):
        eidx = np.where(node_core[dst] == c)[0]
        key = node_slot[dst[eidx]]
        o = np.argsort(key, kind="stable")
        eidx, key = eidx[o], key[o]  # edges sorted by (tile, rank)
        starts = np.searchsorted(key, np.arange(NPAD))  # per-slot run start
        ends = np.searchsorted(key, np.arange(NPAD), side="right")
        used = np.zeros(NPAD, dtype=np.int64)  # consumed edges per node
        for t in range(NT):
            pl = plans[t]
            j = chunk0[t] + pl["K"]  # fixed chunks follow the customs
            for s, w in pl["fixed"]:
                wn = P // s
                nr = t * P + w * wn + np.arange(P) // s  # node of each slot
                eo = used[nr] + np.arange(P) % s
                sel = starts[nr] + eo
                h_s[c, :, j] = h[eidx[sel]].astype(NP_BF16)
                used[t * P + w * wn : t * P + (w + 1) * wn] += s
                j += 1
            # custom chunks: leftover edges, rank order
            nr = t * P + np.arange(P)
            cnt = ends[nr] - starts[nr] - used[nr]
            assert cnt.min() >= 0 and cnt.sum() <= pl["K"] * P
            rel = np.repeat(np.arange(P), cnt)  # dst slot per leftover edge
            if cnt.sum():
                sel = np.concatenate(
                    [
                        starts[nr[r]] + used[nr[r]] + np.arange(cnt[r])
                        for r in range(P)
                    ]
                )
            else:
                sel = np.empty(0, dtype=np.int64)
            for k in range(pl["K"]):
                j = chunk0[t] + k
                lo, hi = k * P, min((k + 1) * P, len(rel))
                n = max(0, hi - lo)
                if n > 0:
                    h_s[c, :n, j] = h[eidx[sel[lo:hi]]].astype(NP_BF16)
                    dstm[c, :n, cust0[t] + k] = rel[lo:hi]

    xT = np.zeros((N_CORES, F, NPAD), dtype=NP_BF16)
    for c in range(N_CORES):
        idsc = node_map[c]
        mm = idsc >= 0
        xT[c][:, mm] = x[idsc[mm]].astype(NP_BF16).T

    # constant one-hot patterns Bs[e, g] = (e//s == g), columns packed
    bpat = np.zeros((P, PAT_COLS), dtype=NP_BF16)
    for s in PAT_SIZES:
        e = np.arange(P)
        bpat[e, PAT_OFF[s] + e // s] = 1.0

    wT = np.ascontiguousarray(W.T.astype(NP_BF16))  # [F+HID, HID]
    bias = np.ascontiguousarray(b.reshape(HID, 1))

    in_maps = [
        {
            "hs": np.ascontiguousarray(h_s[c].reshape(P, Ctot * HID)),
            "dstm": np.ascontiguousarray(dstm[c]),
            "xT": np.ascontiguousarray(xT[c]),
            "bpat": bpat,
            "wT": wT,
            "bias": bias,
        }
        for c in range(N_CORES)
    ]
    return in_maps, plans, node_map


def build(
    plans,
    reps=1,
    loop_reps=1,
    do_hdma=True,
    do_cmp=True,
    do_mm=True,
    do_lin=True,
    do_act=True,
    do_out=True,
    hpair=None,
    ochunk=None,
    hbufs=4,
    hacopy="act",
    halt=False,
):
    hpair = HPAIR if hpair is None else hpair
    ochunk = OCHUNK if ochunk is None else ochunk
    Cs = [pl["K"] + len(pl["fixed"]) for pl in plans]
    Ctot = int(np.sum(Cs))
    ncust = int(np.sum([pl["K"] for pl in plans]))
    f32 = mybir.dt.float32

    nc = bass.Bass()
    hs = nc.dram_tensor("hs", [P, Ctot * HID], BF16, kind="ExternalInput")
    dstm = nc.dram_tensor("dstm", [P, ncust], f32, kind="ExternalInput")
    xT = nc.dram_tensor("xT", [F, NPAD], BF16, kind="ExternalInput")
    bpat_d = nc.dram_tensor("bpat", [P, PAT_COLS], BF16, kind="ExternalInput")
    wT = nc.dram_tensor("wT", [F + HID, HID], BF16, kind="ExternalInput")
    bias = nc.dram_tensor("bias", [HID, 1], f32, kind="ExternalInput")
    outT = nc.dram_tensor("outT", [HID, NPAD], BF16, kind="ExternalOutput")

    with tile.TileContext(nc) as tc, ExitStack() as ctx:
        const = ctx.enter_context(tc.tile_pool(name="const", bufs=1))
        inpool = ctx.enter_context(tc.tile_pool(name="inpool", bufs=2))
        hpool = ctx.enter_context(tc.tile_pool(name="hpool", bufs=hbufs))
        ohpool = ctx.enter_context(tc.tile_pool(name="ohpool", bufs=8))
        hapool = ctx.enter_context(tc.tile_pool(name="hapool", bufs=2))
        opool = ctx.enter_context(tc.tile_pool(name="opool", bufs=2))
        psh = ctx.enter_context(tc.tile_pool(name="psh", bufs=4, space="PSUM"))
        pso = ctx.enter_context(tc.tile_pool(name="pso", bufs=4, space="PSUM"))

        iota_i = const.tile([P, P], mybir.dt.int32)
        nc.gpsimd.iota(iota_i[:], pattern=[[1, P]], base=0, channel_multiplier=0)
        iota_c = const.tile([P, P], BF16)
        nc.vector.tensor_copy(iota_c[:], iota_i[:])

        def body():
            # Per-execution input loads: small tensors ride the ACT HWDGE
            # ring; the big h stream has the SP ring to itself.
            dstm_sb = inpool.tile([P, ncust], f32, tag="dstm")
            nc.scalar.dma_start(dstm_sb[:], dstm[:])
            bp = inpool.tile([P, PAT_COLS], BF16, tag="bp")
            nc.scalar.dma_start(bp[:], bpat_d[:])
            wx = inpool.tile([P, HID], BF16, tag="wx")
            nc.scalar.dma_start(wx[:], wT[0:F, :])
            wh = inpool.tile([P, HID], BF16, tag="wh")
            nc.scalar.dma_start(wh[:], wT[F : F + HID, :])
            bt = inpool.tile([P, 1], f32, tag="bt")
            nc.scalar.dma_start(bt[:], bias[:])
            xall = inpool.tile([P, NPAD], BF16, tag="xall")
            nc.scalar.dma_start(xall[:], xT[:])
            obuf = None
            if (do_act and do_lin) or do_out:
                obuf = opool.tile([P, NPAD], BF16, tag="obuf", name="obuf")

            # h tiles come HPAIR node-tiles per DMA (~1MB) for bandwidth.
            h_group = {}

            def load_group(t0):
                n = sum(Cs[t0 : t0 + hpair])
                j0 = sum(Cs[:t0])
                ht = hpool.tile([P, n * HID], BF16, tag="h_t", name="ht")
                if do_hdma:
                    heng = nc.scalar if (halt and (t0 // hpair) % 2) else nc.sync
                    heng.dma_start(ht[:], hs[:, j0 * HID : (j0 + n) * HID])
                elif do_mm:
                    nc.gpsimd.memset(ht[:, 0:1], 0.0)  # probe: mark written
                for tt in range(t0, min(t0 + hpair, NT)):
                    off = sum(Cs[t0:tt])
                    h_group[tt] = (ht, off)

            ci = 0  # global custom-chunk index
            for t in range(NT):
                pl = plans[t]
                K, fixed = pl["K"], pl["fixed"]
                if t % hpair == 0:
                    load_group(t)
                ht, off = h_group.pop(t)

                def hsl(k, ht=ht, off=off):
                    return ht[:, (off + k) * HID : (off + k + 1) * HID]

                oh = None
                if do_cmp or do_mm:
                    oh = ohpool.tile([P, K * P], BF16, tag="oh", name="oh")
                if do_mm and not do_cmp:
                    nc.gpsimd.memset(oh[:, 0:1], 0.0)  # probe: mark written
                if do_cmp:
                    for k in range(K):
                        # custom one-hot: row e = (iota == dst[e]); bf16
                        # unit-stride operands keep DVE in its fast mode.
                        nc.vector.tensor_scalar(
                            out=oh[:, k * P : (k + 1) * P],
                            in0=iota_c[:],
                            scalar1=dstm_sb[:, ci + k : ci + k + 1],
                            scalar2=None,
                            op0=mybir.AluOpType.is_equal,
                        )
                ph = None
                if do_mm:
                    ph = psh.tile([P, P], f32, tag="ph", name="ph")
                    nmm = K + len(fixed)
                    # custom chunks first: chunk 0 start=True clears the bank
                    # and writes the full 128-column width.
                    for k in range(K):
                        nc.tensor.matmul(
                            out=ph[:],
                            lhsT=hsl(k),
                            rhs=oh[:, k * P : (k + 1) * P],
                            start=(k == 0),
                            stop=(k == nmm - 1),
                        )
                    for i, (s, w) in enumerate(fixed):
                        wn = P // s
                        nc.tensor.matmul(
                            out=ph[:, w * wn : (w + 1) * wn],
                            lhsT=hsl(K + i),
                            rhs=bp[:, PAT_OFF[s] : PAT_OFF[s] + wn],
                            start=False,
                            stop=(K + i == nmm - 1),
                        )
                ci += K
                hA = None
                if do_act and do_mm:
                    hA = hapool.tile([P, P], BF16, tag="hA", name="hA")
                    if hacopy == "dve":
                        nc.vector.tensor_copy(hA[:], ph[:])
                    else:
                        nc.scalar.copy(hA[:], ph[:])
                po = None
                if do_lin:
                    po = pso.tile([P, P], f32, tag="po", name="po")
                    nc.tensor.matmul(
                        out=po[:],
                        lhsT=wx[:],
                        rhs=xall[:, t * P : (t + 1) * P],
                        start=True,
                        stop=False,
                    )
                    rhs2 = hA if (do_act and do_mm) else xall[:, t * P : (t + 1) * P]
                    nc.tensor.matmul(
                        out=po[:], lhsT=wh[:], rhs=rhs2, start=False, stop=True
                    )
                if do_act and do_lin:
                    nc.scalar.activation(
                        obuf[:, t * P : (t + 1) * P],
                        po[:],
                        mybir.ActivationFunctionType.Relu,
                        bias=bt[:, :1],
                    )
                elif do_out:
                    nc.scalar.copy(
                        obuf[:, t * P : (t + 1) * P], xall[:, t * P : (t + 1) * P]
                    )
                if do_out and ((t + 1) % ochunk == 0 or t == NT - 1):
                    lo = (t // ochunk) * ochunk
                    nc.scalar.dma_start(
                        outT[:, lo * P : (t + 1) * P], obuf[:, lo * P : (t + 1) * P]
                    )

        for _rep in range(reps):
            if loop_reps > 1:
                with tc.For_i(0, loop_reps):
                    body()
            else:
                body()
    return nc


def postprocess(results, node_map):
    out = np.empty((N_NODES, HID), dtype=np.float32)
    for c in range(N_CORES):
        ids = node_map[c]
        mask = ids >= 0
        out[ids[mask]] = results[c]["outT"].astype(np.float32).T[mask]
    return out


def kernel(x, h, edge_dst, W, b, **_kw):
    in_maps, plans, node_map = preprocess(x, h, edge_dst, W, b)
    nc = build(plans)
    _split_excess_waits(nc)  # HW-only pass (the sim race detector rejects it)
    results = None
    last_err = None
    for _attempt in range(3):  # device occasionally reports a transient
        try:  # NRT_EXEC_UNIT_UNRECOVERABLE right after a heavy prior session
            res = run_bass_kernel_spmd(nc, in_maps, list(range(N_CORES)))
            results = res.results
            break
        except ModuleNotFoundError:
            # trace path needs antenv.axon_hooks, absent in trimmed clients
            from concourse import bass2jax

            results = bass2jax.run_bass_via_pjrt(nc, in_maps, n_cores=N_CORES)
            break
        except Exception as e:  # noqa: BLE001
            last_err = e
            if "UNRECOVERABLE" not in str(e) and "UNAVAILABLE" not in str(e):
                raise
            import time as _time

            _time.sleep(10)
    if results is None:
        raise last_err
    return postprocess(results, node_map)
